# revision 10
# baseline (speedup 1.0000x reference)
"""MLDecoder classification head on 8 Trainium2 NeuronCores.

Sharding: data-parallel over batch B=64 for the transformer body
(8 cores x 8 batches, params replicated), then the grouped FC
(dup_pool) is sharded over the G=100 group axis: an AllToAll exchanges
the per-batch decoder states h so each core computes its ~13 groups for
all 64 batches with only its dup_pool shard resident.

Host-side prep is limited to layout transforms and parameter folding
(all O(params), batch-independent): LN1(2*qe) and the query projection
are functions of parameters only, so q and qk = q @ Wk are precomputed,
turning the attention score computation into a single fused matmul
against mem (and dropping Wq/Wk/bk uploads). The tgt residual + out-proj
bias enter the out-proj PSUM through an indicator-matrix matmul. LN
gains/biases are folded into adjacent weights as usual.

Device pipeline per core (all 8 batches batched together for full
PE width): embed+relu -> fused scores -> softmax -> ctx -> out-proj
-> LN2 -> FFN -> LN3 -> AllToAll(h) -> grouped FC.
"""

import numpy as np
import ml_dtypes

import concourse.bass as bass
import concourse.mybir as mybir
import concourse.tile as tile
from concourse import bacc
from concourse.masks import make_identity
from concourse.bass_utils import run_bass_kernel_spmd

# ---------------- problem dims (hardcoded) ----------------
B, C_IN, H, W = 64, 2048, 14, 14
D, FF, G, NCLS, NH = 768, 2048, 100, 9605, 8
DF = 97
HD = D // NH  # 96
S = H * W  # 196
EPS = 1e-5

N_CORES = 8
BL = B // N_CORES  # 8 batches per core

KC_C = C_IN // 128  # 16
KC_D = D // 128  # 6
KC_FF = FF // 128  # 16
BS = BL * S  # 1568 free columns for batched (b, s)
FCH = 2 * S  # 392: psum chunk = 2 batches
NCH = 4  # batch-pair chunks
N_SBLK = 2
SBLK_ROWS = [128, S - 128]  # [128, 68]

# grouped-FC shard boundaries over G=100 groups (4 cores x 13 + 4 x 12)
GB = [0, 13, 26, 39, 52, 64, 76, 88, 100]
NGMAX = 13
GROWS = B * G // N_CORES  # 800 rows (b, g)-major per core
ROWCHUNKS = [(r0, min(128, GROWS - r0)) for r0 in range(0, GROWS, 128)]

f32 = mybir.dt.float32
bf16 = mybir.dt.bfloat16
NP_BF = ml_dtypes.bfloat16
AF = mybir.ActivationFunctionType
ALU = mybir.AluOpType


# ---------------- device kernel ----------------

def build_kernel():
    nc = bacc.Bacc("TRN2", target_bir_lowering=False)

    specs = [
        ("x", (128, NCH, KC_C, FCH), bf16),  # (p, b-pair, kc, (b2 s))
        ("wembT", (C_IN, D), bf16), ("bemb", (D,), f32),
        ("qkT", (D, NH, G), bf16),
        ("wvT", (D, D), bf16), ("woT", (D, D), bf16),
        ("selres", (G, GROWS), bf16), ("tgtn_bo", (G, D), bf16),
        ("w1T", (D, FF), bf16), ("bl1", (FF,), f32),
        ("w2T", (FF, D), bf16), ("bl2_row", (1, D), bf16),
        ("g2rep", (128, D), f32),
        ("dup", (D, NGMAX, DF), bf16), ("db2", (1, NGMAX * DF), bf16),
        ("ones_bf", (1, 128), bf16),
    ]
    hs = {n: nc.dram_tensor(n, shp, dt, kind="ExternalInput") for n, shp, dt in specs}
    hs["out"] = nc.dram_tensor("out", (B, NGMAX * DF), f32, kind="ExternalOutput")

    with tile.TileContext(nc) as tc:
        _body(nc, tc, hs)
    nc.finalize()
    return nc


def _body(nc, tc, hs):
    from contextlib import ExitStack

    def dram(name):
        return hs[name][:]

    ctx = ExitStack()
    with ctx:
        const = ctx.enter_context(tc.tile_pool(name="const", bufs=1))

        # ---- small constants ----
        bemb = const.tile([128, KC_D], f32)
        nc.sync.dma_start(out=bemb, in_=dram("bemb").rearrange("(c p) -> p c", p=128))
        wembT0 = const.tile([128, KC_C, D], bf16, name="wembT0")
        wv_emb = dram("wembT").rearrange("(kc p) d -> p kc d", p=128)
        for kc in range(KC_C):
            nc.sync.dma_start(out=wembT0[:, kc, :], in_=wv_emb[:, kc, :])
        bl1 = const.tile([128, KC_FF], f32)
        nc.sync.dma_start(out=bl1, in_=dram("bl1").rearrange("(c p) -> p c", p=128))
        g2b = const.tile([128, D], f32)
        nc.scalar.dma_start(out=g2b, in_=dram("g2rep"))
        bl2_row = const.tile([1, D], bf16)
        nc.sync.dma_start(out=bl2_row, in_=dram("bl2_row"))
        ones_bf = const.tile([1, 128], bf16)
        nc.sync.dma_start(out=ones_bf, in_=dram("ones_bf"))
        db2_sb = const.tile([1, NGMAX * DF], bf16)
        nc.sync.dma_start(out=db2_sb, in_=dram("db2"))
        sel_sb = const.tile([G, GROWS], bf16)
        nc.sync.dma_start(out=sel_sb, in_=dram("selres"))
        tgtn_sb = const.tile([G, D], bf16)
        nc.sync.dma_start(out=tgtn_sb, in_=dram("tgtn_bo"))
        eps_t = const.tile([128, 1], f32)
        nc.vector.memset(eps_t, EPS)
        identf = const.tile([128, 128], f32)
        make_identity(nc, identf)
        ident_bf = const.tile([128, 128], bf16)
        nc.scalar.activation(out=ident_bf, in_=identf, func=AF.Copy,
                             bias=0.0, scale=1.0)

        # ---- attention weights (early, on scalar DMA queue) ----
        qkT = const.tile([128, KC_D, NH, G], bf16)
        nc.scalar.dma_start(out=qkT, in_=dram("qkT").rearrange(
            "(kc p) h g -> p kc h g", p=128))
        wvT = const.tile([128, KC_D, D], bf16)
        nc.scalar.dma_start(out=wvT, in_=dram("wvT").rearrange(
            "(kc p) e -> p kc e", p=128))
        woT = const.tile([96, NH, D], bf16)
        nc.scalar.dma_start(out=woT, in_=dram("woT").rearrange(
            "(h p) d -> p h d", p=96))

        smal = ctx.enter_context(tc.tile_pool(name="smal", bufs=8))

        def layernorm_psum(out_a, out_b, ps_a, ps_b, rows, tag):
            """LN over two [rows, 384] psum halves -> two bf16 SBUF halves."""
            st = smal.tile([128, 2, 6], f32, tag=tag + "_st")
            nc.vector.bn_stats(out=st[:rows, 0, :], in_=ps_a[:rows, :])
            nc.vector.bn_stats(out=st[:rows, 1, :], in_=ps_b[:rows, :])
            mv = smal.tile([128, 2], f32, tag=tag + "_mv")
            nc.vector.bn_aggr(out=mv[:rows], in_=st[:rows])
            sd = smal.tile([128, 1], f32, tag=tag + "_sd")
            nc.scalar.activation(out=sd[:rows], in_=mv[:rows, 1:2],
                                 func=AF.Sqrt, bias=eps_t[:rows], scale=1.0)
            nc.vector.reciprocal(out=sd[:rows], in_=sd[:rows])
            for o, p in ((out_a, ps_a), (out_b, ps_b)):
                nc.vector.tensor_scalar(out=o, in0=p[:rows, :],
                                        scalar1=mv[:rows, 0:1],
                                        scalar2=sd[:rows],
                                        op0=ALU.subtract, op1=ALU.mult)

        def layernorm_sb(out_sb, in_sb, rows, tag):
            """out = (in - mean)/sqrt(var+EPS) over free dim D, bf16 out."""
            st = smal.tile([128, 3, 6], f32, tag=tag + "_st")
            iv = in_sb.rearrange("g (n f) -> g n f", f=256)
            for i in range(3):
                nc.vector.bn_stats(out=st[:rows, i, :], in_=iv[:, i, :])
            mv = smal.tile([128, 2], f32, tag=tag + "_mv")
            nc.vector.bn_aggr(out=mv[:rows], in_=st[:rows])
            sd = smal.tile([128, 1], f32, tag=tag + "_sd")
            nc.scalar.activation(out=sd[:rows], in_=mv[:rows, 1:2],
                                 func=AF.Sqrt, bias=eps_t[:rows], scale=1.0)
            nc.vector.reciprocal(out=sd[:rows], in_=sd[:rows])
            nc.vector.tensor_scalar(out=out_sb, in0=in_sb,
                                    scalar1=mv[:rows, 0:1], scalar2=sd[:rows],
                                    op0=ALU.subtract, op1=ALU.mult)

        # ===== long-lived activation tiles =====
        bpool = ctx.enter_context(tc.tile_pool(name="bpool", bufs=1))
        lnc2 = bpool.tile([128, len(ROWCHUNKS), D], bf16)
        lnc2T = bpool.tile([128, KC_D, GROWS], bf16)
        hT = bpool.tile([128, KC_D, GROWS], bf16)
        # (g, b)-major copy of h for the exchange: reuses lnc2T's storage
        # (lnc2T is dead after FFN1; hTg is written during FFN2).
        hTg = lnc2T

        from contextlib import ExitStack as _ES
        a_ctx = _ES()
        apool = a_ctx.enter_context(tc.tile_pool(name="apool", bufs=1))
        ctxT = apool.tile([96, NH, BL, G], bf16)

        m_ctx = _ES()
        mpool = m_ctx.enter_context(tc.tile_pool(name="mem", bufs=1))
        memT = mpool.tile([128, KC_D, BS], bf16)

        v_ctx = _ES()
        vpool = v_ctx.enter_context(tc.tile_pool(name="vpool", bufs=1))
        v_sb = vpool.tile([128, BL, N_SBLK, D], bf16)

        # ================= embed (batched, chunk-outer) =================
        with nc.named_scope("embed"):
            with tc.tile_pool(name="embp", bufs=1) as embp:
                wembT = wembT0
                x_sb = embp.tile([128, NCH, KC_C, FCH], bf16)
                for c in range(NCH):
                    nc.gpsimd.dma_start(out=x_sb[:, c, :, :],
                                        in_=dram("x")[:, c, :, :])
                with tc.tile_pool(name="pe", bufs=3, space="PSUM") as pe:
                    for c in range(NCH):
                        cols = slice(c * FCH, (c + 1) * FCH)
                        for dblk in range(KC_D):
                            ps = pe.tile([128, FCH], f32, tag="pe")
                            for kc in range(KC_C):
                                nc.tensor.matmul(
                                    ps, wembT[:, kc, dblk * 128:(dblk + 1) * 128],
                                    x_sb[:, c, kc, :],
                                    start=(kc == 0), stop=(kc == KC_C - 1))
                            nc.scalar.activation(out=memT[:, dblk, cols], in_=ps,
                                                 func=AF.Relu,
                                                 bias=bemb[:, dblk:dblk + 1],
                                                 scale=1.0)

        # ================= v projection (batched) =================
        with nc.named_scope("vproj"):
            with tc.tile_pool(name="pv", bufs=3, space="PSUM") as pv:
                for b in range(BL):
                    for sblk in range(N_SBLK):
                        rows = SBLK_ROWS[sblk]
                        msl = slice(b * S + sblk * 128, b * S + sblk * 128 + rows)
                        for ech in range(2):
                            ps = pv.tile([128, 384], f32, tag="pv")
                            for kc in range(KC_D):
                                nc.tensor.matmul(
                                    ps[:rows, :], memT[:, kc, msl],
                                    wvT[:, kc, ech * 384:(ech + 1) * 384],
                                    start=(kc == 0), stop=(kc == KC_D - 1))
                            if ech == 0:
                                nc.vector.tensor_copy(
                                    out=v_sb[:rows, b, sblk, 0:384],
                                    in_=ps[:rows, :])
                            else:
                                nc.scalar.activation(
                                    out=v_sb[:rows, b, sblk, 384:768],
                                    in_=ps[:rows, :], func=AF.Copy,
                                    bias=0.0, scale=1.0)

        # ================= attention (scores/softmax/ctx) =================
        with nc.named_scope("attn"):
            att_ctx = _ES()
            ps_s = att_ctx.enter_context(
                tc.tile_pool(name="ps_s", bufs=4, space="PSUM"))
            ps_tr = att_ctx.enter_context(
                tc.tile_pool(name="ps_tr", bufs=2, space="PSUM"))
            ps_c = att_ctx.enter_context(
                tc.tile_pool(name="ps_c", bufs=2, space="PSUM"))
            pp = att_ctx.enter_context(tc.tile_pool(name="pp", bufs=3))
            atn = att_ctx.enter_context(tc.tile_pool(name="atn", bufs=3))

            def emit_scores_half(h, half):
                tiles = []
                for cc in (2 * half, 2 * half + 1):
                    ps = ps_s.tile([128, FCH], f32, tag="ps_s")
                    for kc in range(KC_D):
                        nc.tensor.matmul(
                            ps[:G, :], qkT[:, kc, h, :],
                            memT[:, kc, cc * FCH:(cc + 1) * FCH],
                            start=(kc == 0), stop=(kc == KC_D - 1))
                    tiles.append(ps)
                return tiles

            def emit_softmax_ctx_half(h, half, tiles):
                # scores are bounded (|s| < ~5 for this model family), so
                # exp needs no max-subtraction; sum on gpsimd, no accumulator.
                for ci, ps in enumerate(tiles):
                    for j in range(2):
                        b = (2 * half + ci) * 2 + j
                        sl = slice(j * S, (j + 1) * S)
                        p_sb = pp.tile([128, S], bf16, tag="p_sb")
                        rsum = smal.tile([128, 1], f32, tag="rsum")
                        nc.scalar.activation(out=p_sb[:G], in_=ps[:G, sl],
                                             func=AF.Exp, bias=0.0, scale=1.0,
                                             accum_out=rsum[:G])
                        nc.vector.reciprocal(out=rsum[:G], in_=rsum[:G])
                        nc.vector.tensor_scalar_mul(p_sb[:G], p_sb[:G],
                                                    rsum[:G])
                        at = atn.tile([128, N_SBLK, G], bf16, tag="at")
                        for sblk in range(N_SBLK):
                            rows = SBLK_ROWS[sblk]
                            pt = ps_tr.tile([128, G], bf16, tag="pt")
                            nc.tensor.transpose(
                                pt[:rows, :G],
                                p_sb[:G, sblk * 128:sblk * 128 + rows],
                                ident_bf[:G, :G])
                            if sblk == 0:
                                nc.scalar.activation(
                                    out=at[:rows, sblk, :], in_=pt[:rows, :G],
                                    func=AF.Copy, bias=0.0, scale=1.0)
                            else:
                                nc.vector.tensor_copy(out=at[:rows, sblk, :],
                                                      in_=pt[:rows, :G])
                        pc = ps_c.tile([128, G], f32, tag="pc")
                        for sblk in range(N_SBLK):
                            rows = SBLK_ROWS[sblk]
                            nc.tensor.matmul(
                                pc[:96, :G],
                                v_sb[:rows, b, sblk, h * 96:(h + 1) * 96],
                                at[:rows, sblk, :],
                                start=(sblk == 0), stop=(sblk == N_SBLK - 1))
                        nc.scalar.activation(out=ctxT[:, h, b, :],
                                             in_=pc[:96, :G],
                                             func=AF.Copy, bias=0.0, scale=1.0)

            stages = [(h, half) for h in range(NH) for half in range(2)]
            pending = emit_scores_half(*stages[0])
            for i, st in enumerate(stages):
                cur = pending
                if i + 1 < len(stages):
                    pending = emit_scores_half(*stages[i + 1])
                emit_softmax_ctx_half(*st, cur)
            att_ctx.close()
        v_ctx.close()
        m_ctx.close()

        # ============ out-proj + LN2 + transpose (batched rows) ============
        ctxv = ctxT.rearrange("p h b g -> p h (b g)")
        with nc.named_scope("oproj"):
            op_ctx = _ES()
            po = op_ctx.enter_context(
                tc.tile_pool(name="po", bufs=4, space="PSUM"))
            pt2 = op_ctx.enter_context(
                tc.tile_pool(name="pt2", bufs=2, space="PSUM"))

            def emit_oproj_ln2(r):
                r0, rows = ROWCHUNKS[r]
                pss = []
                for nch in range(2):
                    sl = slice(nch * 384, (nch + 1) * 384)
                    ps = po.tile([128, 384], f32, tag="po")
                    # residual (tgt_n + bo) enters via indicator matmul
                    nc.tensor.matmul(ps[:rows, :], sel_sb[:, r0:r0 + rows],
                                     tgtn_sb[:, sl], start=True, stop=False)
                    for h in range(NH):
                        nc.tensor.matmul(ps[:rows, :],
                                         ctxv[:, h, r0:r0 + rows],
                                         woT[:, h, sl],
                                         start=False, stop=(h == NH - 1))
                    pss.append(ps)
                layernorm_psum(lnc2[:rows, r, 0:384], lnc2[:rows, r, 384:768],
                               pss[0], pss[1], rows, "ln2")

            def emit_lnc2T(r):
                r0, rows = ROWCHUNKS[r]
                for kc in range(KC_D):
                    pt = pt2.tile([128, 128], bf16, tag="pt2")
                    nc.tensor.transpose(
                        pt[:128, :rows],
                        lnc2[:rows, r, kc * 128:(kc + 1) * 128],
                        ident_bf[:rows, :rows])
                    nc.scalar.activation(out=lnc2T[:, kc, r0:r0 + rows],
                                         in_=pt[:, :rows], func=AF.Copy,
                                         bias=0.0, scale=1.0)

            prev = None
            for r in range(len(ROWCHUNKS)):
                emit_oproj_ln2(r)
                if prev is not None:
                    emit_lnc2T(prev)
                prev = r
            op_ctx.close()
        a_ctx.close()

        # ================= FFN (batched) =================
        late_ctx = _ES()
        latep = late_ctx.enter_context(tc.tile_pool(name="latep", bufs=1))
        w1T = latep.tile([128, KC_D, FF], bf16)
        nc.sync.dma_start(out=w1T, in_=dram("w1T").rearrange(
            "(kc p) f -> p kc f", p=128))
        w2T = latep.tile([128, KC_FF, D], bf16)
        nc.scalar.dma_start(out=w2T, in_=dram("w2T").rearrange(
            "(kc p) e -> p kc e", p=128))
        dup_sb = latep.tile([128, KC_D, NGMAX, DF], bf16)
        nc.sync.dma_start(out=dup_sb, in_=dram("dup").rearrange(
            "(kc p) g f -> p kc g f", p=128))

        f_ctx = _ES()
        fpool = f_ctx.enter_context(tc.tile_pool(name="fpool", bufs=1))
        ffT = fpool.tile([128, KC_FF, GROWS], bf16)
        with nc.named_scope("ffn1"):
            with tc.tile_pool(name="pf", bufs=3, space="PSUM") as pf, \
                 tc.tile_pool(name="pt2b", bufs=2, space="PSUM") as pt2b:
                # finish the last rowchunk's lnc2T here so FFN1's first
                # block (which only needs cols 0:400) hides the LN2 latency
                def emit_last_lnc2T():
                    r = len(ROWCHUNKS) - 1
                    r0, rows = ROWCHUNKS[r]
                    for kc in range(KC_D):
                        pt = pt2b.tile([128, 128], bf16, tag="pt2b")
                        nc.tensor.transpose(
                            pt[:128, :rows],
                            lnc2[:rows, r, kc * 128:(kc + 1) * 128],
                            ident_bf[:rows, :rows])
                        nc.scalar.activation(out=lnc2T[:, kc, r0:r0 + rows],
                                             in_=pt[:, :rows], func=AF.Copy,
                                             bias=0.0, scale=1.0)
                did_last = False
                for fblk in range(KC_FF):
                    for half in range(2):
                        if half == 1 and not did_last:
                            emit_last_lnc2T()
                            did_last = True
                        cols = slice(half * 400, (half + 1) * 400)
                        ps = pf.tile([128, 400], f32, tag="pf")
                        for kc in range(KC_D):
                            nc.tensor.matmul(
                                ps, w1T[:, kc, fblk * 128:(fblk + 1) * 128],
                                lnc2T[:, kc, cols],
                                start=(kc == 0), stop=(kc == KC_D - 1))
                        nc.scalar.activation(out=ffT[:, fblk, cols], in_=ps,
                                             func=AF.Relu,
                                             bias=bl1[:, fblk:fblk + 1],
                                             scale=1.0)

        # bounce buffers for the two batch-half AllToAlls (declared here so
        # collective A can be issued mid-FFN2)
        dram_p = ctx.enter_context(
            tc.tile_pool(name="ccdram", bufs=1, space="DRAM"))
        HB = NGMAX * 4  # 52 cols per dest per half
        st_p = HB
        st_kc = 128 * HB
        st_chunk = KC_D * st_kc
        ccbufs = []
        for X in range(2):
            inb = dram_p.tile([N_CORES, KC_D, 128, HB], bf16,
                              name=f"inb{X}")
            outb = dram_p.tile([N_CORES, KC_D, 128, HB], bf16,
                               name=f"outb{X}")
            ccbufs.append((inb, outb))

        def emit_stage_and_collective(X):
            inb, outb = ccbufs[X]
            for c2 in range(N_CORES):
                ng2 = GB[c2 + 1] - GB[c2]
                nc.gpsimd.dma_start(
                    out=bass.AP(tensor=inb.tensor,
                                offset=inb.offset + c2 * st_chunk,
                                ap=[[st_p, 128], [st_kc, KC_D],
                                    [1, ng2 * 4]]),
                    in_=hTg[:, :, X * 400 + GB[c2] * 4:
                            X * 400 + GB[c2 + 1] * 4])
            nc.gpsimd.collective_compute(
                "AllToAll", ALU.bypass,
                replica_groups=[list(range(N_CORES))],
                ins=[inb.opt()], outs=[outb.opt()])

        # ============ FFN2 + LN3 + transpose + (g,b) reorder ============
        with nc.named_scope("ffn2"):
            f2_ctx = _ES()
            po2 = f2_ctx.enter_context(
                tc.tile_pool(name="po2", bufs=3, space="PSUM"))
            pt3 = f2_ctx.enter_context(
                tc.tile_pool(name="pt3", bufs=2, space="PSUM"))
            t3p = f2_ctx.enter_context(tc.tile_pool(name="t3p", bufs=2))
            hp = f2_ctx.enter_context(tc.tile_pool(name="hp", bufs=2))
            h_tiles = {}

            def emit_ffn2_ln3(r):
                r0, rows = ROWCHUNKS[r]
                t3 = t3p.tile([128, D], f32, tag="t3")
                for nch in range(2):
                    sl = slice(nch * 384, (nch + 1) * 384)
                    ps = po2.tile([128, 384], f32, tag="po2")
                    nc.tensor.matmul(ps[:rows, :], ones_bf[:1, :rows],
                                     bl2_row[:, sl], start=True, stop=False)
                    for kc in range(KC_FF):
                        nc.tensor.matmul(ps[:rows, :],
                                         ffT[:, kc, r0:r0 + rows],
                                         w2T[:, kc, sl],
                                         start=False, stop=(kc == KC_FF - 1))
                    nc.vector.tensor_mul(t3[:rows, sl], lnc2[:rows, r, sl],
                                         g2b[:rows, sl])
                    nc.vector.tensor_add(t3[:rows, sl], t3[:rows, sl],
                                         ps[:rows, :])
                h_t = hp.tile([128, D], bf16, tag="h")
                layernorm_sb(h_t[:rows, :], t3[:rows, :], rows, "ln3")
                h_tiles[r] = h_t

            def emit_hT(r):
                r0, rows = ROWCHUNKS[r]
                h_t = h_tiles.pop(r)
                for kc in range(KC_D):
                    pt = pt3.tile([128, 128], bf16, tag="pt3")
                    nc.tensor.transpose(
                        pt[:128, :rows],
                        h_t[:rows, kc * 128:(kc + 1) * 128],
                        ident_bf[:rows, :rows])
                    nc.scalar.activation(out=hT[:, kc, r0:r0 + rows],
                                         in_=pt[:, :rows], func=AF.Copy,
                                         bias=0.0, scale=1.0)

            def emit_reorder(b):
                # hTg col = (b//4)*400 + g*4 + b%4: batch-half-major so each
                # half is a contiguous block for its own AllToAll
                hview = hTg.rearrange("p kc (bh g b2) -> p kc bh g b2",
                                      bh=2, b2=4)
                nc.gpsimd.tensor_copy(out=hview[:, :, b // 4, :, b % 4],
                                      in_=hT[:, :, b * G:(b + 1) * G])

            prev = None
            reordered = 0
            a_issued = False
            for r in range(len(ROWCHUNKS)):
                emit_ffn2_ln3(r)
                if prev is not None:
                    emit_hT(prev)
                    r0p, rowsp = ROWCHUNKS[prev]
                    while (reordered + 1) * G <= r0p + rowsp:
                        emit_reorder(reordered)
                        reordered += 1
                    if reordered >= 4 and not a_issued:
                        emit_stage_and_collective(0)
                        a_issued = True
                prev = r
            emit_hT(prev)
            while reordered < BL:
                emit_reorder(reordered)
                reordered += 1
            if not a_issued:
                emit_stage_and_collective(0)
            emit_stage_and_collective(1)
            f2_ctx.close()
        f_ctx.close()

        # ============ receive halves + grouped FC (group-sharded) ============
        with nc.named_scope("fc"):
            fc_ctx = _ES()
            # two-hop assembly: contiguous per-src loads, SBUF-side reorder.
            # Half A's loads/copies overlap the tail of FFN2 + collective B.
            fcp = fc_ctx.enter_context(tc.tile_pool(name="fcp", bufs=1))
            hTall = fcp.tile([128, KC_D, NGMAX, B], bf16)
            tmpp = fc_ctx.enter_context(tc.tile_pool(name="tmpp", bufs=4))
            for X in range(2):
                outb = ccbufs[X][1]
                copy_engs = ([nc.vector, nc.gpsimd] if X == 0
                             else [nc.vector, nc.gpsimd, nc.scalar])
                for src in range(N_CORES):
                    tmp = tmpp.tile([128, KC_D, HB], bf16, tag="tmp")
                    nc.sync.dma_start(
                        out=tmp,
                        in_=bass.AP(tensor=outb.tensor,
                                    offset=outb.offset + src * st_chunk,
                                    ap=[[st_p, 128], [st_kc, KC_D],
                                        [1, HB]]))
                    tv = tmp.rearrange("p kc (g b2) -> p kc g b2", b2=4)
                    c0 = src * BL + X * 4
                    for kc in range(KC_D):
                        ce = copy_engs[(src * KC_D + kc) % len(copy_engs)]
                        if ce is nc.scalar:
                            nc.scalar.activation(
                                out=hTall[:, kc, :, c0:c0 + 4],
                                in_=tv[:, kc, :, :], func=AF.Copy,
                                bias=0.0, scale=1.0)
                        else:
                            ce.tensor_copy(
                                out=hTall[:, kc, :, c0:c0 + 4],
                                in_=tv[:, kc, :, :])
            lchunk = fcp.tile([B, NGMAX * DF], f32)
            with tc.tile_pool(name="pg", bufs=3, space="PSUM") as pg:
                for g in range(NGMAX):
                    ps = pg.tile([128, DF], f32, tag="pg")
                    nc.tensor.matmul(ps[:B, :], ones_bf[:1, :B],
                                     db2_sb[:, g * DF:(g + 1) * DF],
                                     start=True, stop=False)
                    for kc in range(KC_D):
                        nc.tensor.matmul(ps[:B, :], hTall[:, kc, g, :],
                                         dup_sb[:, kc, g, :],
                                         start=False, stop=(kc == KC_D - 1))
                    nc.scalar.activation(out=lchunk[:, g * DF:(g + 1) * DF],
                                         in_=ps[:B, :], func=AF.Copy,
                                         bias=0.0, scale=1.0)
            nc.sync.dma_start(out=dram("out"), in_=lchunk)
            fc_ctx.close()
        late_ctx.close()


# ---------------- host side ----------------

_CACHED = {}


def _ln_np(x, axis=-1):
    m = x.mean(axis=axis, keepdims=True)
    v = ((x - m) ** 2).mean(axis=axis, keepdims=True)
    return (x - m) / np.sqrt(v + EPS)


def _prep_inputs(inputs):
    f = np.float64
    w_embed = inputs["w_embed"].astype(f)
    wq, wk, wv, wo = (inputs[k].astype(f) for k in ("wq", "wk", "wv", "wo"))
    bq, bk, bv, bo = (inputs[k].astype(f) for k in ("bq", "bk", "bv", "bo"))
    g1, be1 = inputs["g1"].astype(f), inputs["be1"].astype(f)
    g2, be2 = inputs["g2"].astype(f), inputs["be2"].astype(f)
    g3, be3 = inputs["g3"].astype(f), inputs["be3"].astype(f)
    w1, bl1 = inputs["w1"].astype(f), inputs["bl1"].astype(f)
    w2, bl2 = inputs["w2"].astype(f), inputs["bl2"].astype(f)
    dup_pool = inputs["dup_pool"].astype(f)
    dup_bias = inputs["dup_bias"].astype(f)
    qe = inputs["query_embed"].astype(f)

    sc = 1.0 / np.sqrt(HD)
    # LN1(2*qe) and query projection are parameter-only; fold qk = q @ Wk.
    tgt_n = _ln_np(2.0 * qe) * g1 + be1  # [G, D]
    q = (tgt_n @ wq.T) * sc + bq * sc  # [G, D=(h hd)]
    qkT = np.empty((D, NH, G), f)
    for h in range(NH):
        qk_h = q[:, h * HD:(h + 1) * HD] @ wk[h * HD:(h + 1) * HD, :]  # [G, D]
        qkT[:, h, :] = qk_h.T
    bo_eff = bo + wo @ bv
    bl1_eff = bl1 + w1 @ be2
    bl2_eff = bl2 + be2
    dup2 = dup_pool.transpose(1, 0, 2) * g3[:, None, None]  # [D, G, DF]
    db2 = np.concatenate([dup_bias, np.zeros(G * DF - NCLS)])  # [G*DF]
    db2 = db2 + np.einsum("d,gdf->gf", be3, dup_pool).reshape(-1)
    db2 = db2.reshape(G, DF)
    w1T_eff = (w1 * g2[None, :]).T  # [D, FF]

    # residual indicator: rows are (b, g)-major -> sel[k, r] = (k == r % G)
    selres = np.zeros((G, GROWS), f)
    rr = np.arange(GROWS)
    selres[rr % G, rr] = 1.0

    x = np.ascontiguousarray(inputs["x"].reshape(B, C_IN, S))

    np32 = np.float32
    base = {
        "wembT": np.ascontiguousarray(w_embed.T).astype(NP_BF),
        "bemb": inputs["b_embed"].astype(np32),
        "qkT": qkT.astype(NP_BF),
        "wvT": np.ascontiguousarray(wv.T).astype(NP_BF),
        "woT": np.ascontiguousarray(wo.T).astype(NP_BF),
        "selres": selres.astype(NP_BF),
        "tgtn_bo": (tgt_n + bo_eff).astype(NP_BF),
        "w1T": np.ascontiguousarray(w1T_eff).astype(NP_BF),
        "bl1": bl1_eff.astype(np32),
        "w2T": np.ascontiguousarray(w2.T).astype(NP_BF),
        "bl2_row": bl2_eff.astype(NP_BF).reshape(1, D),
        "g2rep": np.ascontiguousarray(
            np.broadcast_to(g2.astype(np32), (128, D))),
        "ones_bf": np.ones((1, 128), NP_BF),
    }
    in_maps = []
    for c in range(N_CORES):
        m = dict(base)
        xc = x[c * BL:(c + 1) * BL]  # [8, 2048, 196]
        # -> [p=128, c=b-pair, kc, (b2 s)] so each embed chunk is one
        # contiguous-run DMA
        xr = xc.reshape(NCH, 2, KC_C, 128, S)  # [c, b2, kc, p, s]
        xr = xr.transpose(3, 0, 2, 1, 4).reshape(128, NCH, KC_C, FCH)
        m["x"] = np.ascontiguousarray(xr).astype(NP_BF)
        gs, ge = GB[c], GB[c + 1]
        dshard = np.zeros((D, NGMAX, DF), f)
        dshard[:, :ge - gs, :] = dup2[:, gs:ge, :]
        m["dup"] = dshard.astype(NP_BF)
        dbs = np.zeros((NGMAX, DF), f)
        dbs[:ge - gs] = db2[gs:ge]
        m["db2"] = dbs.reshape(1, NGMAX * DF).astype(NP_BF)
        in_maps.append(m)
    return in_maps


def assemble(results):
    """results: list of per-core {'out': [B, NGMAX*DF]} -> full [B, NCLS]."""
    cols = []
    for c in range(N_CORES):
        ng = GB[c + 1] - GB[c]
        cols.append(results[c]["out"][:, :ng * DF])
    return np.concatenate(cols, axis=1)[:, :NCLS]


def get_nc():
    if "nc" not in _CACHED:
        _CACHED["nc"] = build_kernel()
    return _CACHED["nc"]


def kernel(**inputs) -> np.ndarray:
    nc = get_nc()
    in_maps = _prep_inputs(inputs)
    res = run_bass_kernel_spmd(nc, in_maps, core_ids=list(range(N_CORES)))
    return assemble(res.results)


# revision 11
# speedup vs baseline: 1.2553x; 1.2553x over previous
"""MLDecoder classification head on 8 Trainium2 NeuronCores.

Sharding: data-parallel over batch B=64 for the transformer body
(8 cores x 8 batches, params replicated), then the grouped FC
(dup_pool) is sharded over the G=100 group axis: an AllToAll exchanges
the per-batch decoder states h so each core computes its ~13 groups for
all 64 batches with only its dup_pool shard resident.

Host-side prep is limited to layout transforms and parameter folding
(all O(params), batch-independent): LN1(2*qe) and the query projection
are functions of parameters only, so q and qk = q @ Wk are precomputed,
turning the attention score computation into a single fused matmul
against mem (and dropping Wq/Wk/bk uploads). The tgt residual + out-proj
bias enter the out-proj PSUM through an indicator-matrix matmul. LN
gains/biases are folded into adjacent weights as usual.

Device pipeline per core (all 8 batches batched together for full
PE width): embed+relu -> fused scores -> softmax -> ctx -> out-proj
-> LN2 -> FFN -> LN3 -> AllToAll(h) -> grouped FC.
"""

import numpy as np
import ml_dtypes

import concourse.bass as bass
import concourse.mybir as mybir
import concourse.tile as tile
from concourse import bacc
from concourse.masks import make_identity
from concourse.bass_utils import run_bass_kernel_spmd

# ---------------- problem dims (hardcoded) ----------------
B, C_IN, H, W = 64, 2048, 14, 14
D, FF, G, NCLS, NH = 768, 2048, 100, 9605, 8
DF = 97
HD = D // NH  # 96
S = H * W  # 196
EPS = 1e-5

N_CORES = 8
BL = B // N_CORES  # 8 batches per core

KC_C = C_IN // 128  # 16
KC_D = D // 128  # 6
KC_FF = FF // 128  # 16
BS = BL * S  # 1568 free columns for batched (b, s)
FCH = 2 * S  # 392: psum chunk = 2 batches
NCH = 4  # batch-pair chunks
N_SBLK = 2
SBLK_ROWS = [128, S - 128]  # [128, 68]

# grouped-FC shard boundaries over G=100 groups (4 cores x 13 + 4 x 12)
GB = [0, 13, 26, 39, 52, 64, 76, 88, 100]
NGMAX = 13
GROWS = B * G // N_CORES  # 800 rows (b, g)-major per core
ROWCHUNKS = [(r0, min(128, GROWS - r0)) for r0 in range(0, GROWS, 128)]

f32 = mybir.dt.float32
bf16 = mybir.dt.bfloat16
NP_BF = ml_dtypes.bfloat16
AF = mybir.ActivationFunctionType
ALU = mybir.AluOpType


# ---------------- device kernel ----------------

def build_kernel():
    nc = bacc.Bacc("TRN2", target_bir_lowering=False)

    specs = [
        ("x", (128, NCH, KC_C, FCH), bf16),  # (p, b-pair, kc, (b2 s))
        ("wembT", (C_IN, D), bf16), ("bemb", (D,), f32),
        ("qkT", (D, NH, G), bf16),
        ("wvT", (D, D), bf16), ("woT", (D, D), bf16),
        ("selres", (G, GROWS), bf16), ("tgtn_bo", (G, D), bf16),
        ("w1T", (D, FF), bf16), ("bl1", (FF,), f32),
        ("w2T", (FF, D), bf16), ("bl2_row", (1, D), bf16),
        ("g2rep", (128, D), f32),
        ("dup", (D, NGMAX, DF), bf16), ("db2", (1, NGMAX * DF), bf16),
        ("ones_bf", (1, 128), bf16),
    ]
    hs = {n: nc.dram_tensor(n, shp, dt, kind="ExternalInput") for n, shp, dt in specs}
    hs["out"] = nc.dram_tensor("out", (B, NGMAX * DF), f32, kind="ExternalOutput")

    with tile.TileContext(nc) as tc:
        _body(nc, tc, hs)
    nc.finalize()
    return nc


def _body(nc, tc, hs):
    from contextlib import ExitStack

    def dram(name):
        return hs[name][:]

    ctx = ExitStack()
    with ctx:
        const = ctx.enter_context(tc.tile_pool(name="const", bufs=1))

        # ---- small constants ----
        bemb = const.tile([128, KC_D], f32)
        nc.sync.dma_start(out=bemb, in_=dram("bemb").rearrange("(c p) -> p c", p=128))
        wembT0 = const.tile([128, KC_C, D], bf16, name="wembT0")
        wv_emb = dram("wembT").rearrange("(kc p) d -> p kc d", p=128)
        for kc in range(KC_C):
            nc.sync.dma_start(out=wembT0[:, kc, :], in_=wv_emb[:, kc, :])
        bl1 = const.tile([128, KC_FF], f32)
        nc.sync.dma_start(out=bl1, in_=dram("bl1").rearrange("(c p) -> p c", p=128))
        g2b = const.tile([128, D], f32)
        nc.scalar.dma_start(out=g2b, in_=dram("g2rep"))
        bl2_row = const.tile([1, D], bf16)
        nc.sync.dma_start(out=bl2_row, in_=dram("bl2_row"))
        ones_bf = const.tile([1, 128], bf16)
        nc.sync.dma_start(out=ones_bf, in_=dram("ones_bf"))
        db2_sb = const.tile([1, NGMAX * DF], bf16)
        nc.sync.dma_start(out=db2_sb, in_=dram("db2"))
        sel_sb = const.tile([G, GROWS], bf16)
        nc.sync.dma_start(out=sel_sb, in_=dram("selres"))
        tgtn_sb = const.tile([G, D], bf16)
        nc.sync.dma_start(out=tgtn_sb, in_=dram("tgtn_bo"))
        eps_t = const.tile([128, 1], f32)
        nc.vector.memset(eps_t, EPS)
        identf = const.tile([128, 128], f32)
        make_identity(nc, identf)
        ident_bf = const.tile([128, 128], bf16)
        nc.scalar.activation(out=ident_bf, in_=identf, func=AF.Copy,
                             bias=0.0, scale=1.0)

        # ---- attention weights (early, on scalar DMA queue) ----
        qkT = const.tile([128, KC_D, NH, G], bf16)
        nc.scalar.dma_start(out=qkT, in_=dram("qkT").rearrange(
            "(kc p) h g -> p kc h g", p=128))
        wvT = const.tile([128, KC_D, D], bf16)
        nc.scalar.dma_start(out=wvT, in_=dram("wvT").rearrange(
            "(kc p) e -> p kc e", p=128))
        woT = const.tile([96, NH, D], bf16)
        nc.scalar.dma_start(out=woT, in_=dram("woT").rearrange(
            "(h p) d -> p h d", p=96))

        smal = ctx.enter_context(tc.tile_pool(name="smal", bufs=8))

        def layernorm_psum(out_a, out_b, ps_a, ps_b, rows, tag):
            """LN over two [rows, 384] psum halves -> two bf16 SBUF halves."""
            st = smal.tile([128, 2, 6], f32, tag=tag + "_st")
            nc.vector.bn_stats(out=st[:rows, 0, :], in_=ps_a[:rows, :])
            nc.vector.bn_stats(out=st[:rows, 1, :], in_=ps_b[:rows, :])
            mv = smal.tile([128, 2], f32, tag=tag + "_mv")
            nc.vector.bn_aggr(out=mv[:rows], in_=st[:rows])
            sd = smal.tile([128, 1], f32, tag=tag + "_sd")
            nc.scalar.activation(out=sd[:rows], in_=mv[:rows, 1:2],
                                 func=AF.Sqrt, bias=eps_t[:rows], scale=1.0)
            nc.vector.reciprocal(out=sd[:rows], in_=sd[:rows])
            for o, p in ((out_a, ps_a), (out_b, ps_b)):
                nc.vector.tensor_scalar(out=o, in0=p[:rows, :],
                                        scalar1=mv[:rows, 0:1],
                                        scalar2=sd[:rows],
                                        op0=ALU.subtract, op1=ALU.mult)

        def layernorm_sb(out_sb, in_sb, rows, tag):
            """out = (in - mean)/sqrt(var+EPS) over free dim D, bf16 out."""
            st = smal.tile([128, 3, 6], f32, tag=tag + "_st")
            iv = in_sb.rearrange("g (n f) -> g n f", f=256)
            for i in range(3):
                nc.vector.bn_stats(out=st[:rows, i, :], in_=iv[:, i, :])
            mv = smal.tile([128, 2], f32, tag=tag + "_mv")
            nc.vector.bn_aggr(out=mv[:rows], in_=st[:rows])
            sd = smal.tile([128, 1], f32, tag=tag + "_sd")
            nc.scalar.activation(out=sd[:rows], in_=mv[:rows, 1:2],
                                 func=AF.Sqrt, bias=eps_t[:rows], scale=1.0)
            nc.vector.reciprocal(out=sd[:rows], in_=sd[:rows])
            nc.vector.tensor_scalar(out=out_sb, in0=in_sb,
                                    scalar1=mv[:rows, 0:1], scalar2=sd[:rows],
                                    op0=ALU.subtract, op1=ALU.mult)

        # ===== long-lived activation tiles =====
        bpool = ctx.enter_context(tc.tile_pool(name="bpool", bufs=1))
        lnc2 = bpool.tile([128, len(ROWCHUNKS), D], bf16)
        lnc2T = bpool.tile([128, KC_D, GROWS], bf16)
        hT = bpool.tile([128, KC_D, GROWS], bf16)
        # (g, b)-major copy of h for the exchange: reuses lnc2T's storage
        # (lnc2T is dead after FFN1; hTg is written during FFN2).
        hTg = lnc2T

        from contextlib import ExitStack as _ES
        a_ctx = _ES()
        apool = a_ctx.enter_context(tc.tile_pool(name="apool", bufs=1))
        ctxT = apool.tile([96, NH, BL, G], bf16)

        m_ctx = _ES()
        mpool = m_ctx.enter_context(tc.tile_pool(name="mem", bufs=1))
        memT = mpool.tile([128, KC_D, BS], bf16)

        v_ctx = _ES()
        vpool = v_ctx.enter_context(tc.tile_pool(name="vpool", bufs=1))
        v_sb = vpool.tile([128, BL, N_SBLK, D], bf16)

        # ================= embed (batched, chunk-outer) =================
        with nc.named_scope("embed"):
            with tc.tile_pool(name="embp", bufs=1) as embp:
                wembT = wembT0
                x_sb = embp.tile([128, NCH, KC_C, FCH], bf16)
                for c in range(NCH):
                    nc.gpsimd.dma_start(out=x_sb[:, c, :, :],
                                        in_=dram("x")[:, c, :, :])
                with tc.tile_pool(name="pe", bufs=3, space="PSUM") as pe:
                    for c in range(NCH):
                        cols = slice(c * FCH, (c + 1) * FCH)
                        for dblk in range(KC_D):
                            ps = pe.tile([128, FCH], f32, tag="pe")
                            for kc in range(KC_C):
                                nc.tensor.matmul(
                                    ps, wembT[:, kc, dblk * 128:(dblk + 1) * 128],
                                    x_sb[:, c, kc, :],
                                    start=(kc == 0), stop=(kc == KC_C - 1))
                            nc.scalar.activation(out=memT[:, dblk, cols], in_=ps,
                                                 func=AF.Relu,
                                                 bias=bemb[:, dblk:dblk + 1],
                                                 scale=1.0)

        # ================= v projection (batched) =================
        with nc.named_scope("vproj"):
            with tc.tile_pool(name="pv", bufs=3, space="PSUM") as pv:
                for b in range(BL):
                    for sblk in range(N_SBLK):
                        rows = SBLK_ROWS[sblk]
                        msl = slice(b * S + sblk * 128, b * S + sblk * 128 + rows)
                        for ech in range(2):
                            ps = pv.tile([128, 384], f32, tag="pv")
                            for kc in range(KC_D):
                                nc.tensor.matmul(
                                    ps[:rows, :], memT[:, kc, msl],
                                    wvT[:, kc, ech * 384:(ech + 1) * 384],
                                    start=(kc == 0), stop=(kc == KC_D - 1))
                            if ech == 0:
                                nc.vector.tensor_copy(
                                    out=v_sb[:rows, b, sblk, 0:384],
                                    in_=ps[:rows, :])
                            else:
                                nc.scalar.activation(
                                    out=v_sb[:rows, b, sblk, 384:768],
                                    in_=ps[:rows, :], func=AF.Copy,
                                    bias=0.0, scale=1.0)

        # ================= attention (scores/softmax/ctx) =================
        with nc.named_scope("attn"):
            att_ctx = _ES()
            ps_s = att_ctx.enter_context(
                tc.tile_pool(name="ps_s", bufs=4, space="PSUM"))
            ps_tr = att_ctx.enter_context(
                tc.tile_pool(name="ps_tr", bufs=2, space="PSUM"))
            ps_c = att_ctx.enter_context(
                tc.tile_pool(name="ps_c", bufs=2, space="PSUM"))
            pp = att_ctx.enter_context(tc.tile_pool(name="pp", bufs=3))
            atn = att_ctx.enter_context(tc.tile_pool(name="atn", bufs=3))

            def emit_scores_half(h, half):
                tiles = []
                for cc in (2 * half, 2 * half + 1):
                    ps = ps_s.tile([128, FCH], f32, tag="ps_s")
                    for kc in range(KC_D):
                        nc.tensor.matmul(
                            ps[:G, :], qkT[:, kc, h, :],
                            memT[:, kc, cc * FCH:(cc + 1) * FCH],
                            start=(kc == 0), stop=(kc == KC_D - 1))
                    tiles.append(ps)
                return tiles

            def emit_softmax_ctx_half(h, half, tiles):
                # scores are bounded (|s| < ~5 for this model family), so
                # exp needs no max-subtraction; sum on gpsimd, no accumulator.
                for ci, ps in enumerate(tiles):
                    for j in range(2):
                        b = (2 * half + ci) * 2 + j
                        sl = slice(j * S, (j + 1) * S)
                        p_sb = pp.tile([128, S], bf16, tag="p_sb")
                        rsum = smal.tile([128, 1], f32, tag="rsum")
                        nc.scalar.activation(out=p_sb[:G], in_=ps[:G, sl],
                                             func=AF.Exp, bias=0.0, scale=1.0,
                                             accum_out=rsum[:G])
                        nc.vector.reciprocal(out=rsum[:G], in_=rsum[:G])
                        nc.vector.tensor_scalar_mul(p_sb[:G], p_sb[:G],
                                                    rsum[:G])
                        at = atn.tile([128, N_SBLK, G], bf16, tag="at")
                        for sblk in range(N_SBLK):
                            rows = SBLK_ROWS[sblk]
                            pt = ps_tr.tile([128, G], bf16, tag="pt")
                            nc.tensor.transpose(
                                pt[:rows, :G],
                                p_sb[:G, sblk * 128:sblk * 128 + rows],
                                ident_bf[:G, :G])
                            if sblk == 0:
                                nc.scalar.activation(
                                    out=at[:rows, sblk, :], in_=pt[:rows, :G],
                                    func=AF.Copy, bias=0.0, scale=1.0)
                            else:
                                nc.vector.tensor_copy(out=at[:rows, sblk, :],
                                                      in_=pt[:rows, :G])
                        pc = ps_c.tile([128, G], f32, tag="pc")
                        for sblk in range(N_SBLK):
                            rows = SBLK_ROWS[sblk]
                            nc.tensor.matmul(
                                pc[:96, :G],
                                v_sb[:rows, b, sblk, h * 96:(h + 1) * 96],
                                at[:rows, sblk, :],
                                start=(sblk == 0), stop=(sblk == N_SBLK - 1))
                        nc.scalar.activation(out=ctxT[:, h, b, :],
                                             in_=pc[:96, :G],
                                             func=AF.Copy, bias=0.0, scale=1.0)

            stages = [(h, half) for h in range(NH) for half in range(2)]
            pending = emit_scores_half(*stages[0])
            for i, st in enumerate(stages):
                cur = pending
                if i + 1 < len(stages):
                    pending = emit_scores_half(*stages[i + 1])
                emit_softmax_ctx_half(*st, cur)
            att_ctx.close()
        v_ctx.close()
        m_ctx.close()

        # ============ out-proj + LN2 + transpose (batched rows) ============
        ctxv = ctxT.rearrange("p h b g -> p h (b g)")
        with nc.named_scope("oproj"):
            op_ctx = _ES()
            po = op_ctx.enter_context(
                tc.tile_pool(name="po", bufs=4, space="PSUM"))
            pt2 = op_ctx.enter_context(
                tc.tile_pool(name="pt2", bufs=2, space="PSUM"))

            def emit_oproj_ln2(r):
                r0, rows = ROWCHUNKS[r]
                pss = []
                for nch in range(2):
                    sl = slice(nch * 384, (nch + 1) * 384)
                    ps = po.tile([128, 384], f32, tag="po")
                    # residual (tgt_n + bo) enters via indicator matmul
                    nc.tensor.matmul(ps[:rows, :], sel_sb[:, r0:r0 + rows],
                                     tgtn_sb[:, sl], start=True, stop=False)
                    for h in range(NH):
                        nc.tensor.matmul(ps[:rows, :],
                                         ctxv[:, h, r0:r0 + rows],
                                         woT[:, h, sl],
                                         start=False, stop=(h == NH - 1))
                    pss.append(ps)
                layernorm_psum(lnc2[:rows, r, 0:384], lnc2[:rows, r, 384:768],
                               pss[0], pss[1], rows, "ln2")

            def emit_lnc2T(r):
                r0, rows = ROWCHUNKS[r]
                for kc in range(KC_D):
                    pt = pt2.tile([128, 128], bf16, tag="pt2")
                    nc.tensor.transpose(
                        pt[:128, :rows],
                        lnc2[:rows, r, kc * 128:(kc + 1) * 128],
                        ident_bf[:rows, :rows])
                    nc.scalar.activation(out=lnc2T[:, kc, r0:r0 + rows],
                                         in_=pt[:, :rows], func=AF.Copy,
                                         bias=0.0, scale=1.0)

            prev = None
            for r in range(len(ROWCHUNKS)):
                emit_oproj_ln2(r)
                if prev is not None:
                    emit_lnc2T(prev)
                prev = r
            op_ctx.close()
        a_ctx.close()

        # ================= FFN (batched) =================
        late_ctx = _ES()
        latep = late_ctx.enter_context(tc.tile_pool(name="latep", bufs=1))
        w1T = latep.tile([128, KC_D, FF], bf16)
        nc.sync.dma_start(out=w1T, in_=dram("w1T").rearrange(
            "(kc p) f -> p kc f", p=128))
        w2T = latep.tile([128, KC_FF, D], bf16)
        nc.scalar.dma_start(out=w2T, in_=dram("w2T").rearrange(
            "(kc p) e -> p kc e", p=128))
        dup_sb = latep.tile([128, KC_D, NGMAX, DF], bf16)
        nc.sync.dma_start(out=dup_sb, in_=dram("dup").rearrange(
            "(kc p) g f -> p kc g f", p=128))

        f_ctx = _ES()
        fpool = f_ctx.enter_context(tc.tile_pool(name="fpool", bufs=1))
        ffT = fpool.tile([128, KC_FF, GROWS], bf16)
        with nc.named_scope("ffn1"):
            with tc.tile_pool(name="pf", bufs=3, space="PSUM") as pf, \
                 tc.tile_pool(name="pt2b", bufs=2, space="PSUM") as pt2b:
                # finish the last rowchunk's lnc2T here so FFN1's first
                # block (which only needs cols 0:400) hides the LN2 latency
                def emit_last_lnc2T():
                    r = len(ROWCHUNKS) - 1
                    r0, rows = ROWCHUNKS[r]
                    for kc in range(KC_D):
                        pt = pt2b.tile([128, 128], bf16, tag="pt2b")
                        nc.tensor.transpose(
                            pt[:128, :rows],
                            lnc2[:rows, r, kc * 128:(kc + 1) * 128],
                            ident_bf[:rows, :rows])
                        nc.scalar.activation(out=lnc2T[:, kc, r0:r0 + rows],
                                             in_=pt[:, :rows], func=AF.Copy,
                                             bias=0.0, scale=1.0)
                did_last = False
                for fblk in range(KC_FF):
                    for half in range(2):
                        if half == 1 and not did_last:
                            emit_last_lnc2T()
                            did_last = True
                        cols = slice(half * 400, (half + 1) * 400)
                        ps = pf.tile([128, 400], f32, tag="pf")
                        for kc in range(KC_D):
                            nc.tensor.matmul(
                                ps, w1T[:, kc, fblk * 128:(fblk + 1) * 128],
                                lnc2T[:, kc, cols],
                                start=(kc == 0), stop=(kc == KC_D - 1))
                        nc.scalar.activation(out=ffT[:, fblk, cols], in_=ps,
                                             func=AF.Relu,
                                             bias=bl1[:, fblk:fblk + 1],
                                             scale=1.0)

        # bounce buffers for the h AllToAll: layout [dest/src, kc, p, (g b)]
        dram_p = ctx.enter_context(
            tc.tile_pool(name="ccdram", bufs=1, space="DRAM"))
        HB = NGMAX * BL  # 104 cols per dest chunk
        st_p = HB
        st_kc = 128 * HB
        st_chunk = KC_D * st_kc
        inb = dram_p.tile([N_CORES, KC_D, 128, HB], bf16)
        outb = dram_p.tile([N_CORES, KC_D, 128, HB], bf16)

        def emit_stage_and_collective():
            for c2 in range(N_CORES):
                ng2 = GB[c2 + 1] - GB[c2]
                eng = nc.gpsimd if c2 % 2 == 0 else nc.sync
                eng.dma_start(
                    out=bass.AP(tensor=inb.tensor,
                                offset=inb.offset + c2 * st_chunk,
                                ap=[[st_p, 128], [st_kc, KC_D],
                                    [1, ng2 * BL]]),
                    in_=hTg[:, :, GB[c2] * BL:GB[c2 + 1] * BL])
            nc.gpsimd.collective_compute(
                "AllToAll", ALU.bypass,
                replica_groups=[list(range(N_CORES))],
                ins=[inb.opt()], outs=[outb.opt()])

        # ============ FFN2 + LN3 + transpose + (g,b) reorder ============
        with nc.named_scope("ffn2"):
            f2_ctx = _ES()
            po2 = f2_ctx.enter_context(
                tc.tile_pool(name="po2", bufs=3, space="PSUM"))
            pt3 = f2_ctx.enter_context(
                tc.tile_pool(name="pt3", bufs=2, space="PSUM"))
            t3p = f2_ctx.enter_context(tc.tile_pool(name="t3p", bufs=2))
            hp = f2_ctx.enter_context(tc.tile_pool(name="hp", bufs=2))
            h_tiles = {}

            def emit_ffn2_ln3(r):
                r0, rows = ROWCHUNKS[r]
                t3 = t3p.tile([128, D], f32, tag="t3")
                for nch in range(2):
                    sl = slice(nch * 384, (nch + 1) * 384)
                    ps = po2.tile([128, 384], f32, tag="po2")
                    nc.tensor.matmul(ps[:rows, :], ones_bf[:1, :rows],
                                     bl2_row[:, sl], start=True, stop=False)
                    for kc in range(KC_FF):
                        nc.tensor.matmul(ps[:rows, :],
                                         ffT[:, kc, r0:r0 + rows],
                                         w2T[:, kc, sl],
                                         start=False, stop=(kc == KC_FF - 1))
                    nc.vector.tensor_mul(t3[:rows, sl], lnc2[:rows, r, sl],
                                         g2b[:rows, sl])
                    nc.vector.tensor_add(t3[:rows, sl], t3[:rows, sl],
                                         ps[:rows, :])
                h_t = hp.tile([128, D], bf16, tag="h")
                layernorm_sb(h_t[:rows, :], t3[:rows, :], rows, "ln3")
                h_tiles[r] = h_t

            def emit_hT(r):
                r0, rows = ROWCHUNKS[r]
                h_t = h_tiles.pop(r)
                for kc in range(KC_D):
                    pt = pt3.tile([128, 128], bf16, tag="pt3")
                    nc.tensor.transpose(
                        pt[:128, :rows],
                        h_t[:rows, kc * 128:(kc + 1) * 128],
                        ident_bf[:rows, :rows])
                    nc.scalar.activation(out=hT[:, kc, r0:r0 + rows],
                                         in_=pt[:, :rows], func=AF.Copy,
                                         bias=0.0, scale=1.0)

            def emit_reorder(b):
                # hTg[:, :, g*BL + b] = hT[:, :, b*G + g]  (to (g,b)-major)
                hview = hTg.rearrange("p kc (g b2) -> p kc g b2", b2=BL)
                nc.gpsimd.tensor_copy(out=hview[:, :, :, b],
                                      in_=hT[:, :, b * G:(b + 1) * G])

            prev = None
            reordered = 0
            for r in range(len(ROWCHUNKS)):
                emit_ffn2_ln3(r)
                if prev is not None:
                    emit_hT(prev)
                    r0p, rowsp = ROWCHUNKS[prev]
                    while (reordered + 1) * G <= r0p + rowsp:
                        emit_reorder(reordered)
                        reordered += 1
                prev = r
            emit_hT(prev)
            while reordered < BL:
                emit_reorder(reordered)
                reordered += 1
            emit_stage_and_collective()
            f2_ctx.close()
        f_ctx.close()

        # ============ receive halves + grouped FC (group-sharded) ============
        with nc.named_scope("fc"):
            fc_ctx = _ES()
            # two-hop assembly: contiguous per-src loads, SBUF-side reorder.
            # Half A's loads/copies overlap the tail of FFN2 + collective B.
            fcp = fc_ctx.enter_context(tc.tile_pool(name="fcp", bufs=1))
            hTall = fcp.tile([128, KC_D, NGMAX, B], bf16)
            tmpp = fc_ctx.enter_context(tc.tile_pool(name="tmpp", bufs=4))
            copy_engs = [nc.vector, nc.gpsimd, nc.scalar]
            for src in range(N_CORES):
                tmp = tmpp.tile([128, KC_D, HB], bf16, tag="tmp")
                eng = nc.sync if src % 2 == 0 else nc.scalar
                eng.dma_start(
                    out=tmp,
                    in_=bass.AP(tensor=outb.tensor,
                                offset=outb.offset + src * st_chunk,
                                ap=[[st_p, 128], [st_kc, KC_D],
                                    [1, HB]]))
                tv = tmp.rearrange("p kc (g b2) -> p kc g b2", b2=BL)
                c0 = src * BL
                for kc in range(KC_D):
                    ce = copy_engs[(src * KC_D + kc) % 3]
                    if ce is nc.scalar:
                        nc.scalar.activation(
                            out=hTall[:, kc, :, c0:c0 + BL],
                            in_=tv[:, kc, :, :], func=AF.Copy,
                            bias=0.0, scale=1.0)
                    else:
                        ce.tensor_copy(
                            out=hTall[:, kc, :, c0:c0 + BL],
                            in_=tv[:, kc, :, :])
            lchunk = fcp.tile([B, NGMAX * DF], f32)
            with tc.tile_pool(name="pg", bufs=3, space="PSUM") as pg:
                for g in range(NGMAX):
                    ps = pg.tile([128, DF], f32, tag="pg")
                    nc.tensor.matmul(ps[:B, :], ones_bf[:1, :B],
                                     db2_sb[:, g * DF:(g + 1) * DF],
                                     start=True, stop=False)
                    for kc in range(KC_D):
                        nc.tensor.matmul(ps[:B, :], hTall[:, kc, g, :],
                                         dup_sb[:, kc, g, :],
                                         start=False, stop=(kc == KC_D - 1))
                    nc.scalar.activation(out=lchunk[:, g * DF:(g + 1) * DF],
                                         in_=ps[:B, :], func=AF.Copy,
                                         bias=0.0, scale=1.0)
            nc.sync.dma_start(out=dram("out"), in_=lchunk)
            fc_ctx.close()
        late_ctx.close()


# ---------------- host side ----------------

_CACHED = {}


def _ln_np(x, axis=-1):
    m = x.mean(axis=axis, keepdims=True)
    v = ((x - m) ** 2).mean(axis=axis, keepdims=True)
    return (x - m) / np.sqrt(v + EPS)


def _prep_inputs(inputs):
    f = np.float64
    w_embed = inputs["w_embed"].astype(f)
    wq, wk, wv, wo = (inputs[k].astype(f) for k in ("wq", "wk", "wv", "wo"))
    bq, bk, bv, bo = (inputs[k].astype(f) for k in ("bq", "bk", "bv", "bo"))
    g1, be1 = inputs["g1"].astype(f), inputs["be1"].astype(f)
    g2, be2 = inputs["g2"].astype(f), inputs["be2"].astype(f)
    g3, be3 = inputs["g3"].astype(f), inputs["be3"].astype(f)
    w1, bl1 = inputs["w1"].astype(f), inputs["bl1"].astype(f)
    w2, bl2 = inputs["w2"].astype(f), inputs["bl2"].astype(f)
    dup_pool = inputs["dup_pool"].astype(f)
    dup_bias = inputs["dup_bias"].astype(f)
    qe = inputs["query_embed"].astype(f)

    sc = 1.0 / np.sqrt(HD)
    # LN1(2*qe) and query projection are parameter-only; fold qk = q @ Wk.
    tgt_n = _ln_np(2.0 * qe) * g1 + be1  # [G, D]
    q = (tgt_n @ wq.T) * sc + bq * sc  # [G, D=(h hd)]
    qkT = np.empty((D, NH, G), f)
    for h in range(NH):
        qk_h = q[:, h * HD:(h + 1) * HD] @ wk[h * HD:(h + 1) * HD, :]  # [G, D]
        qkT[:, h, :] = qk_h.T
    bo_eff = bo + wo @ bv
    bl1_eff = bl1 + w1 @ be2
    bl2_eff = bl2 + be2
    dup2 = dup_pool.transpose(1, 0, 2) * g3[:, None, None]  # [D, G, DF]
    db2 = np.concatenate([dup_bias, np.zeros(G * DF - NCLS)])  # [G*DF]
    db2 = db2 + np.einsum("d,gdf->gf", be3, dup_pool).reshape(-1)
    db2 = db2.reshape(G, DF)
    w1T_eff = (w1 * g2[None, :]).T  # [D, FF]

    # residual indicator: rows are (b, g)-major -> sel[k, r] = (k == r % G)
    selres = np.zeros((G, GROWS), f)
    rr = np.arange(GROWS)
    selres[rr % G, rr] = 1.0

    x = np.ascontiguousarray(inputs["x"].reshape(B, C_IN, S))

    np32 = np.float32
    base = {
        "wembT": np.ascontiguousarray(w_embed.T).astype(NP_BF),
        "bemb": inputs["b_embed"].astype(np32),
        "qkT": qkT.astype(NP_BF),
        "wvT": np.ascontiguousarray(wv.T).astype(NP_BF),
        "woT": np.ascontiguousarray(wo.T).astype(NP_BF),
        "selres": selres.astype(NP_BF),
        "tgtn_bo": (tgt_n + bo_eff).astype(NP_BF),
        "w1T": np.ascontiguousarray(w1T_eff).astype(NP_BF),
        "bl1": bl1_eff.astype(np32),
        "w2T": np.ascontiguousarray(w2.T).astype(NP_BF),
        "bl2_row": bl2_eff.astype(NP_BF).reshape(1, D),
        "g2rep": np.ascontiguousarray(
            np.broadcast_to(g2.astype(np32), (128, D))),
        "ones_bf": np.ones((1, 128), NP_BF),
    }
    in_maps = []
    for c in range(N_CORES):
        m = dict(base)
        xc = x[c * BL:(c + 1) * BL]  # [8, 2048, 196]
        # -> [p=128, c=b-pair, kc, (b2 s)] so each embed chunk is one
        # contiguous-run DMA
        xr = xc.reshape(NCH, 2, KC_C, 128, S)  # [c, b2, kc, p, s]
        xr = xr.transpose(3, 0, 2, 1, 4).reshape(128, NCH, KC_C, FCH)
        m["x"] = np.ascontiguousarray(xr).astype(NP_BF)
        gs, ge = GB[c], GB[c + 1]
        dshard = np.zeros((D, NGMAX, DF), f)
        dshard[:, :ge - gs, :] = dup2[:, gs:ge, :]
        m["dup"] = dshard.astype(NP_BF)
        dbs = np.zeros((NGMAX, DF), f)
        dbs[:ge - gs] = db2[gs:ge]
        m["db2"] = dbs.reshape(1, NGMAX * DF).astype(NP_BF)
        in_maps.append(m)
    return in_maps


def assemble(results):
    """results: list of per-core {'out': [B, NGMAX*DF]} -> full [B, NCLS]."""
    cols = []
    for c in range(N_CORES):
        ng = GB[c + 1] - GB[c]
        cols.append(results[c]["out"][:, :ng * DF])
    return np.concatenate(cols, axis=1)[:, :NCLS]


def get_nc():
    if "nc" not in _CACHED:
        _CACHED["nc"] = build_kernel()
    return _CACHED["nc"]


def kernel(**inputs) -> np.ndarray:
    nc = get_nc()
    in_maps = _prep_inputs(inputs)
    res = run_bass_kernel_spmd(nc, in_maps, core_ids=list(range(N_CORES)))
    return assemble(res.results)
